# revision 10
# baseline (speedup 1.0000x reference)
"""Trainium2 Bass kernel for nn_DagnabbitAutoEncoder (gnn_message_passing).

Strategy (8 NeuronCores, data-parallel over nodes within each topological level):
- The (N, D) embeddings buffer is replicated per core in DRAM, but stored in a
  *permuted* row order: each level's slice is the rank-major concatenation of
  each core's locally type-sorted (and padded) node list. That makes the
  per-level exchange a plain contiguous AllGather with no scatter DMAs; all
  permutations are folded into host-precomputed int32 gather index tables.
- Per level, each core:
    1. indirect-DMA gathers the two parent rows of each of its nodes
       (columns type-sorted; output-node second parents redirected to
       slot-embedding rows appended to the buffer),
    2. PE-transposes the gathered tiles to feature-major,
    3. runs the two-layer per-type MLPs as exact grouped matmuls
       (nodes on the moving free dim, 128-wide K chunks, f32 PSUM accum),
       with Gelu+b1 on ScalarE between layers and b2 added on PSUM evacuation,
    4. transposes back, writes its rows to the AllGather staging slice and to
       the external output, and kicks the AllGather into the shared buffer.
- The final output is assembled host-side from the 8 per-core outputs.
"""

import numpy as np

D = 128
R = 1024
T = 4
OUT_SLOTS = 256
START = T
L = 64
M = 2048
N = R + L * M
NCORES = 8
S = M // NCORES  # 256 nodes per core per level

_P = 128  # partitions


def _preprocess(par, typ):
    """Per-level structure + per-core index tables, with early/late split.

    Early nodes have no parent in the immediately previous level, so their
    gathers+compute overlap the previous level's AllGather; late nodes wait.
    """
    e_idx = np.where(typ >= START, T, typ).astype(np.int64)  # (L, M)
    slot = np.clip(typ - START, 0, OUT_SLOTS - 1).astype(np.int64)

    buf_row = np.full(N, -1, dtype=np.int64)
    buf_row[:R] = np.arange(R)

    levels = []
    loc2pos = []
    off = R
    stage_off = 0
    qbase = 0
    for l in range(L):
        el = e_idx[l].reshape(NCORES, S)
        prev_start = R + (l - 1) * M if l > 0 else 0  # fresh = parent id >= this
        pl = par[l].reshape(NCORES, S, 2)
        late = ((pl[:, :, 0] >= prev_start) | (pl[:, :, 1] >= prev_start)) if l > 0 \
            else np.zeros((NCORES, S), bool)
        cE = np.stack([((el == t) & ~late).sum(axis=1) for t in range(T + 1)], axis=1)
        cL = np.stack([((el == t) & late).sum(axis=1) for t in range(T + 1)], axis=1)
        gE = cE.max(axis=0); gE = gE + (gE % 2)
        gL = cL.max(axis=0); gL = gL + (gL % 2)
        CLE = int(gE.sum()); CLL = int(gL.sum())
        nchE = (CLE + _P - 1) // _P
        nchL = (CLL + _P - 1) // _P
        lstart = nchE * _P
        SL = lstart + CLL if nchL > 0 else CLE
        nch = nchE + nchL
        assert lstart + CLL <= 512, f"level {l}: {lstart + CLL} > 512"

        sE = np.concatenate([[0], np.cumsum(gE)])
        sLt = np.concatenate([[0], np.cumsum(gL)])
        groupsE = [(t, int(sE[t]), int(sE[t + 1])) for t in range(T + 1) if gE[t] > 0]
        groupsL = [(t, lstart + int(sLt[t]), lstart + int(sLt[t + 1]))
                   for t in range(T + 1) if gL[t] > 0]

        lp = np.full((NCORES, SL), -1, dtype=np.int64)
        for c in range(NCORES):
            for t in range(T + 1):
                pe = np.nonzero((el[c] == t) & ~late[c])[0] + c * S
                lp[c, sE[t]:sE[t] + len(pe)] = pe
                if nchL > 0:
                    pll = np.nonzero((el[c] == t) & late[c])[0] + c * S
                    st = lstart + sLt[t]
                    lp[c, st:st + len(pll)] = pll

        for c in range(NCORES):
            real = lp[c] >= 0
            gids = R + l * M + lp[c][real]
            buf_row[gids] = off + c * SL + np.nonzero(real)[0]

        levels.append(dict(CLE=CLE, CLL=CLL, nchE=nchE, nchL=nchL, nch=nch,
                           lstart=lstart, SL=SL, groupsE=groupsE, groupsL=groupsL,
                           qbase=qbase, stage_off=stage_off, buf_off=off))
        off += NCORES * SL
        stage_off += SL
        qbase += 2 * nch

    slots_base = off
    buf_rows = off + OUT_SLOTS
    stage_rows = stage_off
    QTOT = qbase
    assert (buf_row[R:] >= 0).all()

    idx_tab = np.zeros((NCORES, _P, QTOT), dtype=np.int32)
    for l in range(L):
        lv = levels[l]
        qb, nchE, nchL, SL = lv["qbase"], lv["nchE"], lv["nchL"], lv["SL"]
        lp = loc2pos_l = loc2pos[l] if False else None
        lp = np.full((NCORES, lv["nch"] * _P), -1, dtype=np.int64)
        # rebuild padded-to-chunks local positions: E cols [0, nchE*128), L cols after
        # (we stored lp over SL columns; map into chunk space)
        # E columns 0..CLE-1 occupy same positions; dead zone [CLE, lstart); L at lstart..
        # chunk space == column space here because lstart = nchE*128. SL <= nch*128.
        # so just copy
        pass
    # simpler: compute idx tables from stored per-level lp arrays
    loc2pos = []
    for l in range(L):
        lv = levels[l]
        el = e_idx[l].reshape(NCORES, S)
        prev_start = R + (l - 1) * M if l > 0 else 0
        pl = par[l].reshape(NCORES, S, 2)
        late = ((pl[:, :, 0] >= prev_start) | (pl[:, :, 1] >= prev_start)) if l > 0 \
            else np.zeros((NCORES, S), bool)
        gE = np.array([e for e in np.diff([s for (t, s, e) in []])]) if False else None
        SL = lv["SL"]
        lp = np.full((NCORES, SL), -1, dtype=np.int64)
        # recompute group starts
        cE = np.stack([((el == t) & ~late).sum(axis=1) for t in range(T + 1)], axis=1)
        cL = np.stack([((el == t) & late).sum(axis=1) for t in range(T + 1)], axis=1)
        gEv = cE.max(axis=0); gEv = gEv + (gEv % 2)
        gLv = cL.max(axis=0); gLv = gLv + (gLv % 2)
        sE = np.concatenate([[0], np.cumsum(gEv)])
        sLt = np.concatenate([[0], np.cumsum(gLv)])
        for c in range(NCORES):
            for t in range(T + 1):
                pe = np.nonzero((el[c] == t) & ~late[c])[0] + c * S
                lp[c, sE[t]:sE[t] + len(pe)] = pe
                if lv["nchL"] > 0:
                    pll = np.nonzero((el[c] == t) & late[c])[0] + c * S
                    st = lv["lstart"] + sLt[t]
                    lp[c, st:st + len(pll)] = pll
        loc2pos.append(lp)
        qb, nchE, nchL = lv["qbase"], lv["nchE"], lv["nchL"]
        nch = lv["nch"]
        for c in range(NCORES):
            p0 = np.zeros(nch * _P, dtype=np.int64)
            p1 = np.zeros(nch * _P, dtype=np.int64)
            real = lp[c] >= 0
            pos = lp[c][real]
            j = np.nonzero(real)[0]
            v0 = par[l, pos, 0]
            v1 = par[l, pos, 1]
            is_out = typ[l, pos] >= START
            p0[j] = buf_row[v0]
            p1[j] = np.where(is_out, slots_base + slot[l, pos], buf_row[v1])
            # chunk columns: E p0 chunks, E p1 chunks, L p0 chunks, L p1 chunks
            p0c = p0.reshape(nch, _P).T
            p1c = p1.reshape(nch, _P).T
            idx_tab[c, :, qb:qb + nchE] = p0c[:, :nchE]
            idx_tab[c, :, qb + nchE:qb + 2 * nchE] = p1c[:, :nchE]
            if nchL > 0:
                idx_tab[c, :, qb + 2 * nchE:qb + 2 * nchE + nchL] = p0c[:, nchE:]
                idx_tab[c, :, qb + 2 * nchE + nchL:qb + 2 * nch] = p1c[:, nchE:]

    return dict(levels=levels, loc2pos=loc2pos, idx_tab=idx_tab, QTOT=QTOT,
                stage_rows=stage_rows, buf_rows=buf_rows, slots_base=slots_base)


def _build_program(pp):
    import concourse.bass as bass
    import concourse.bacc as bacc
    import concourse.mybir as mybir
    import concourse.tile as tile
    from concourse.masks import make_identity

    if not hasattr(bass.log, "warn_once_per_message"):
        bass.log.warn_once_per_message = bass.log.warn_once_per_call_stack

    f32 = mybir.dt.float32
    f32r = mybir.dt.float32r
    nc = bacc.Bacc("TRN2", target_bir_lowering=False, debug=False,
                   num_devices=NCORES)

    root_p = nc.declare_dram_parameter("root", [R, D], f32, isOutput=False)
    w1_p = nc.declare_dram_parameter("w1", [T + 1, 2 * D, 2 * D], f32, isOutput=False)
    b1_p = nc.declare_dram_parameter("b1", [T + 1, 2 * D], f32, isOutput=False)
    w2_p = nc.declare_dram_parameter("w2", [T + 1, 2 * D, D], f32, isOutput=False)
    b2_p = nc.declare_dram_parameter("b2", [T + 1, D], f32, isOutput=False)
    slots_p = nc.declare_dram_parameter("slots", [OUT_SLOTS, D], f32, isOutput=False)
    idx_p = nc.declare_dram_parameter("idx", [_P, pp["QTOT"]], mybir.dt.int32, isOutput=False)
    out_p = nc.declare_dram_parameter("out", [pp["stage_rows"], D], f32, isOutput=True)

    buf = nc.dram_tensor("buf", [pp["buf_rows"], D], f32, addr_space="Shared")
    stage = nc.dram_tensor("stage", [pp["stage_rows"], D], f32)

    nch_max = max(lv["nch"] for lv in pp["levels"])
    CL_max = max((lv["lstart"] + lv["CLL"]) if lv["nchL"] else lv["CLE"]
                 for lv in pp["levels"])

    with tile.TileContext(nc) as tc:
        with tc.tile_pool(name="const", bufs=1) as const, \
             tc.tile_pool(name="gat", bufs=3) as gat, \
             tc.tile_pool(name="xt", bufs=3) as xt, \
             tc.tile_pool(name="ht", bufs=2) as htp, \
             tc.tile_pool(name="et", bufs=2) as etp, \
             tc.tile_pool(name="esb", bufs=3) as esbp, \
             tc.tile_pool(name="tps", bufs=2, space="PSUM") as tps, \
             tc.tile_pool(name="hps", bufs=1, space="PSUM") as hps, \
             tc.tile_pool(name="eps", bufs=2, space="PSUM") as eps:

            ident = const.tile([_P, _P], f32)
            make_identity(nc, ident[:])

            # constants to SBUF
            idx_t = const.tile([_P, pp["QTOT"]], mybir.dt.int32)
            nc.sync.dma_start(idx_t[:], idx_p[:])
            w1_t = const.tile([_P, (T + 1) * 4, _P], f32)  # [(t,kc,mc)]
            nc.sync.dma_start(
                w1_t[:].rearrange("p (t k m) o -> p t k m o", t=T + 1, k=2),
                w1_p[:].rearrange("t (k p) (m o) -> p t k m o", p=_P, o=_P))
            w2_t = const.tile([_P, (T + 1) * 2, _P], f32)  # [(t,kc)]
            nc.sync.dma_start(
                w2_t[:].rearrange("p (t k) o -> p t k o", t=T + 1),
                w2_p[:].rearrange("t (k p) o -> p t k o", p=_P))
            b1_t = const.tile([_P, (T + 1) * 2], f32)  # [(t,mc)]
            nc.sync.dma_start(
                b1_t[:].rearrange("p (t m) -> p t m", t=T + 1),
                b1_p[:].rearrange("t (m p) -> p t m", p=_P))
            b2_t = const.tile([_P, T + 1], f32)
            nc.sync.dma_start(b2_t[:], b2_p[:].rearrange("t p -> p t"))
            w1_r = const.tile([_P, (T + 1) * 4, _P], f32r)
            nc.vector.tensor_copy(w1_r[:], w1_t[:])
            w2_r = const.tile([_P, (T + 1) * 2, _P], f32r)
            nc.vector.tensor_copy(w2_r[:], w2_t[:])

            # init buf: roots + slot embeddings
            nc.sync.dma_start(
                buf[0:R].rearrange("(a p) d -> p a d", p=_P),
                root_p[:].rearrange("(a p) d -> p a d", p=_P))
            sb = pp["slots_base"]
            nc.sync.dma_start(
                buf[sb:sb + OUT_SLOTS].rearrange("(a p) d -> p a d", p=_P),
                slots_p[:].rearrange("(a p) d -> p a d", p=_P))

            # PE warmup (absorbs identity dep)
            warm = tps.tile([_P, _P], f32, tag="tp")
            nc.tensor.transpose(out=warm[:], in_=ident[:], identity=ident[:])

            gelu = mybir.ActivationFunctionType.Gelu

            gtiles = {}

            def emit_E_gathers(l):
                lv = pp["levels"][l]
                qb, nchE = lv["qbase"], lv["nchE"]
                limE = pp["levels"][l - 1]["buf_off"] if l >= 1 else R
                ga = gat.tile([_P, nch_max, _P], f32, tag="ga", name=f"ga{l}")
                gb = gat.tile([_P, nch_max, _P], f32, tag="gb", name=f"gb{l}")
                gtiles[l] = (ga, gb)
                for q in range(nchE):
                    nc.gpsimd.indirect_dma_start(
                        out=ga[:, q, :], out_offset=None, in_=buf[0:limE],
                        in_offset=bass.IndirectOffsetOnAxis(
                            ap=idx_t[:, qb + q:qb + q + 1], axis=0))
                    nc.gpsimd.indirect_dma_start(
                        out=gb[:, q, :], out_offset=None, in_=buf[0:limE],
                        in_offset=bass.IndirectOffsetOnAxis(
                            ap=idx_t[:, qb + nchE + q:qb + nchE + q + 1], axis=0))

            emit_E_gathers(0)

            for l in range(L):
                lv = pp["levels"][l]
                nchE, nchL, nch = lv["nchE"], lv["nchL"], lv["nch"]
                CLE, CLL, lstart, SL = lv["CLE"], lv["CLL"], lv["lstart"], lv["SL"]
                qb = lv["qbase"]
                st_off, b_off = lv["stage_off"], lv["buf_off"]
                limE = pp["levels"][l - 1]["buf_off"] if l >= 1 else R
                limL = b_off

                ga, gb = gtiles.pop(l)
                p0T = xt.tile([_P, nch_max * _P], f32r, tag="p0T", name=f"p0T{l}")
                p1T = xt.tile([_P, nch_max * _P], f32r, tag="p1T", name=f"p1T{l}")
                h_ps = [hps.tile([_P, CL_max], f32, tag=f"h{mc}", name=f"h{mc}_{l}") for mc in range(2)]
                hT = [htp.tile([_P, CL_max], f32r, tag=f"hT{mc}", name=f"hT{mc}_{l}") for mc in range(2)]
                e_ps = eps.tile([_P, CL_max], f32, tag="e", name=f"e_{l}")
                eT = etp.tile([_P, CL_max], f32, tag="eT", name=f"eT{l}")
                e_sb = esbp.tile([_P, nch_max, _P], f32, tag="e_sb", name=f"e_sb{l}")

                def gather(q, tabcol, lim):
                    nc.gpsimd.indirect_dma_start(
                        out=ga[:, q, :], out_offset=None, in_=buf[0:lim],
                        in_offset=bass.IndirectOffsetOnAxis(
                            ap=idx_t[:, tabcol:tabcol + 1], axis=0))

                def gatherb(q, tabcol, lim):
                    nc.gpsimd.indirect_dma_start(
                        out=gb[:, q, :], out_offset=None, in_=buf[0:lim],
                        in_offset=bass.IndirectOffsetOnAxis(
                            ap=idx_t[:, tabcol:tabcol + 1], axis=0))

                def transp(q):
                    tp = tps.tile([_P, _P], f32, tag="tp", name=f"tp{l}_{q}")
                    nc.tensor.transpose(out=tp[:], in_=ga[:, q, :], identity=ident[:])
                    nc.vector.tensor_copy(p0T[:, q * _P:(q + 1) * _P], tp[:])
                    tp2 = tps.tile([_P, _P], f32, tag="tp", name=f"tq{l}_{q}")
                    nc.tensor.transpose(out=tp2[:], in_=gb[:, q, :], identity=ident[:])
                    nc.vector.tensor_copy(p1T[:, q * _P:(q + 1) * _P], tp2[:])

                def mlp(groups):
                    for (t, s, e) in groups:
                        for kc, srct in ((0, p0T), (1, p1T)):
                            for mc in range(2):
                                nc.tensor.matmul(
                                    h_ps[mc][:, s:e],
                                    lhsT=w1_r[:, (t * 2 + kc) * 2 + mc, :],
                                    rhs=srct[:, s:e],
                                    start=(kc == 0), stop=(kc == 1))
                    for mc in range(2):
                        for (t, s, e) in groups:
                            nc.scalar.activation(
                                hT[mc][:, s:e], h_ps[mc][:, s:e], gelu,
                                bias=b1_t[:, t * 2 + mc:t * 2 + mc + 1], scale=1.0)
                    for (t, s, e) in groups:
                        for kc in range(2):
                            nc.tensor.matmul(
                                e_ps[:, s:e],
                                lhsT=w2_r[:, t * 2 + kc, :],
                                rhs=hT[kc][:, s:e],
                                start=(kc == 0), stop=(kc == 1))
                    for (t, s, e) in groups:
                        nc.vector.tensor_scalar(
                            out=eT[:, s:e], in0=e_ps[:, s:e],
                            scalar1=b2_t[:, t:t + 1], scalar2=None,
                            op0=mybir.AluOpType.add)

                # early part (gathers were emitted during level l-1)
                for q in range(nchE):
                    transp(q)
                mlp(lv["groupsE"])

                # late part: needs AG(l-1)
                for q in range(nchL):
                    gather(nchE + q, qb + 2 * nchE + q, limL)
                    gatherb(nchE + q, qb + 2 * nchE + nchL + q, limL)
                if l + 1 < L:
                    emit_E_gathers(l + 1)
                for q in range(nchL):
                    transp(nchE + q)
                mlp(lv["groupsL"])

                for q in range(nch):
                    rem = min(_P, SL - q * _P)
                    if rem <= 0:
                        continue
                    tp3 = tps.tile([_P, _P], f32, tag="tp", name=f"tr{l}_{q}")
                    nc.tensor.transpose(out=tp3[0:rem, :],
                                        in_=eT[:, q * _P:q * _P + rem],
                                        identity=ident[:])
                    nc.vector.tensor_copy(e_sb[0:rem, q, :], tp3[0:rem, :])

                for q in range(nch):
                    rem = min(_P, SL - q * _P)
                    if rem <= 0:
                        continue
                    nc.sync.dma_start(out_p[st_off + q * _P: st_off + q * _P + rem],
                                      e_sb[0:rem, q, :])
                    if l < L - 1:
                        nc.sync.dma_start(stage[st_off + q * _P: st_off + q * _P + rem],
                                          e_sb[0:rem, q, :])

                if l < L - 1:
                    nc.gpsimd.collective_compute(
                        "AllGather", mybir.AluOpType.bypass,
                        replica_groups=[list(range(NCORES))],
                        ins=[stage[st_off:st_off + SL]],
                        outs=[buf[b_off:b_off + NCORES * SL]])

    nc.compile()
    return nc


def kernel(root_node_embeddings, W1, b1, W2, b2, out_slot_emb,
           node_inputs_indices, node_types, _trace=False):
    from concourse.bass_utils import run_bass_kernel_spmd

    par = np.asarray(node_inputs_indices).astype(np.int64).reshape(L, M, 2)
    typ = np.asarray(node_types).astype(np.int64).reshape(L, M)

    pp = _preprocess(par, typ)
    nc = _build_program(pp)

    common = {
        "root": np.ascontiguousarray(np.asarray(root_node_embeddings, dtype=np.float32)),
        "w1": np.ascontiguousarray(np.asarray(W1, dtype=np.float32)),
        "b1": np.ascontiguousarray(np.asarray(b1, dtype=np.float32)),
        "w2": np.ascontiguousarray(np.asarray(W2, dtype=np.float32)),
        "b2": np.ascontiguousarray(np.asarray(b2, dtype=np.float32)),
        "slots": np.ascontiguousarray(np.asarray(out_slot_emb, dtype=np.float32)),
    }
    in_maps = [dict(common, idx=np.ascontiguousarray(pp["idx_tab"][c]))
               for c in range(NCORES)]

    res = run_bass_kernel_spmd(nc, in_maps, list(range(NCORES)), trace=_trace)

    out_full = np.empty((N, D), dtype=np.float32)
    out_full[:R] = common["root"]
    for l in range(L):
        lv = pp["levels"][l]
        lp = pp["loc2pos"][l]
        for c in range(NCORES):
            rows = res.results[c]["out"][lv["stage_off"]:lv["stage_off"] + lv["SL"]]
            real = lp[c] >= 0
            out_full[R + l * M + lp[c][real]] = rows[real]

    if _trace:
        return out_full, res
    return out_full
